# revision 10
# baseline (speedup 1.0000x reference)
# Multi-head attention (B=2, N=2048, C=1024, H=16) on 8 trn2 NeuronCores.
#
# Sharding: core = (batch b = core//4, head-group hg = core%4, 4 heads each).
# Each core computes qkv/attention/proj for its 4 heads of its batch and
# returns a partial projection output [N, C] in bf16; the host sums the 4
# partials per batch in f32 and adds b_proj.
#
# All matmul operands are bf16 (PSUM accumulation stays f32); measured
# end-to-end rel_absmax ~7e-3 vs the f32 reference.
#
# Per-core device pipeline:
#   0. Host supplies x already transposed (xT [C, N]) and cast to bf16, so
#      there are no PE transposes at all. Weights packed per-core on host.
#   1. qkT[4*128, N] = Wq/k @ x^T  (lhsT=wq slice, rhs=xT chunk). Matmuls
#      use 256-wide moving chunks: measured slice cost is 112 ns vs 278 ns
#      for 512-wide (the PE SBUF-access latency hides under 256-row
#      streams), so 2x the instructions is still ~20% faster.
#   2. v natural [n, u*64+d] = (xT tile)^T @ wv -> v1[j, u, jt, 0:64],
#      col 64 = ones (rowsum trick).
#   3. Attention as ONE flat 64-step software pipeline over (u, jt): S(s)
#      is issued ahead of O(s-1) across unit boundaries too, so the PE
#      never drains at a head switch. E=exp(S*scale) on ACT -> bf16;
#      O'^T[65, N] += v'^T @ E^T (row 64 = rowsum). psO released by a
#      single [65,512] DVE copy per tile; reciprocal (DVE), chunked
#      partition-broadcast (gpsimd), multiply (DVE) trail behind.
#   4. proj partial out[i,:] per 128-row tile: one [128,1024] psum tile,
#      8 256-wide matmuls, one ACT/DVE copy to bf16, one DMA (issued on
#      sync, which is otherwise idle).
import sys

import numpy as np

if "/opt/trn_rl_repo" not in sys.path:
    sys.path.insert(0, "/opt/trn_rl_repo")

B, NSEQ, C = 2, 2048, 1024
H, HD = 16, 64
P = 128
SCALE = HD**-0.5

_cache = {}


def _build(nseq):
    from contextlib import ExitStack

    import concourse.tile as tile
    from concourse import bacc, mybir

    f32 = mybir.dt.float32
    bf16 = mybir.dt.bfloat16
    EXP = mybir.ActivationFunctionType.Exp

    NJT = nseq // P          # j tiles (keys) per head
    NIT = nseq // P          # i tiles
    QCH = min(512, nseq)     # psum O-tile width
    MCH = 256                # matmul moving-dim chunk
    NCH = nseq // QCH        # number of seq chunks in scope A
    SW = min(1024, nseq)     # S^T psum tile width (2 banks)
    NSW = nseq // SW
    NOB = nseq // QCH        # number of O' psum tiles
    VW = 66                  # v1 row width (65 used: 64 v dims + ones col)
    NU = 4                   # heads per core

    nc = bacc.Bacc("TRN2", target_bir_lowering=False, debug=False, num_devices=8)
    xt_d = nc.dram_tensor("xT", [P, 8, nseq], bf16, kind="ExternalInput")
    wq_d = nc.dram_tensor("wq", [P, 8, 4 * P], bf16, kind="ExternalInput")
    wv_d = nc.dram_tensor("wv", [P, 8, 4 * HD], bf16, kind="ExternalInput")
    wp_d = nc.dram_tensor("wp", [P, 2, C], bf16, kind="ExternalInput")
    out_d = nc.dram_tensor("out", [nseq, C], bf16, kind="ExternalOutput")

    cp_state = [0]

    def cp(out, in_):
        # alternate PSUM->SBUF copies between DVE and ACT
        cp_state[0] ^= 1
        if cp_state[0]:
            nc.vector.tensor_copy(out, in_)
        else:
            nc.scalar.copy(out, in_)

    with tile.TileContext(nc) as tc, ExitStack() as ctx:
        persist = ctx.enter_context(tc.tile_pool(name="persist", bufs=1))

        # ---- input DMAs, priority order: wq by mt, xT chunk 0, rest ----
        wq_sb = persist.tile([P, 8, 4 * P], bf16)
        xt_sb = persist.tile([P, 8, nseq], bf16)
        wv_sb = persist.tile([P, 8, 4 * HD], bf16)
        wp_sb = persist.tile([P, 2, C], bf16)
        nc.sync.dma_start(wq_sb[:, :, 0:2 * P], wq_d[:, :, 0:2 * P])
        nc.scalar.dma_start(wq_sb[:, :, 2 * P:4 * P], wq_d[:, :, 2 * P:4 * P])
        dma_engines = [nc.sync, nc.scalar, nc.gpsimd]
        for co in range(8):  # first seq chunk, highest priority
            dma_engines[co % 3].dma_start(
                xt_sb[:, co, 0:QCH], xt_d[:, co, 0:QCH]
            )
        nc.gpsimd.dma_start(wv_sb, wv_d.ap())
        for co in range(8):  # rest of the sequence
            dma_engines[co % 3].dma_start(
                xt_sb[:, co, QCH:nseq], xt_d[:, co, QCH:nseq]
            )
        nc.gpsimd.dma_start(wp_sb, wp_d.ap())

        # q^T/k^T per head, zero-padded to full 128 partitions.
        # slot u = q of head u; slot 4+u = k of head u.
        qk_sb = persist.tile([P, 8, nseq], bf16)
        # v' natural [j_part, u, jt, 0:64]=v, col 64 = ones.
        v1 = persist.tile([P, NU, NJT, VW], bf16)
        for slot in range(8):
            zpb = 64 if slot % 2 == 0 else 0
            nc.vector.memset(qk_sb[zpb : zpb + 64, slot, :], 0.0)
        nc.vector.memset(v1[:, :, :, HD : HD + 1], 1.0)

        # prime the ACT exp table early so head 0 doesn't stall on it
        ones_f32 = persist.tile([P, 1], f32)
        nc.vector.memset(ones_f32, 1.0)
        prime = persist.tile([P, 1], f32)
        nc.scalar.activation(prime, ones_f32, EXP, scale=0.0)

        # ======== scope A: qk matmuls + v-natural build ========
        # mt 0 = q heads (0,1), mt 1 = q heads (2,3), mt 2 = k (0,1),
        # mt 3 = k (2,3). psQ partitions 0:64 = first head of pair.
        MT_SLOTS = [(0, 1), (2, 3), (4, 5), (6, 7)]
        with (
            tc.tile_pool(name="psQ", bufs=3, space="PSUM") as psQ,
            tc.tile_pool(name="psV", bufs=3, space="PSUM") as psV,
        ):
            for ch in range(NCH):
                sl = slice(ch * QCH, (ch + 1) * QCH)
                for mt in range(4):
                    ps = psQ.tile([P, QCH], f32, tag="psQ")
                    # m outer: one open accumulation group per psum bank at
                    # a time (interleaved open groups corrupt accumulation)
                    for m in range(QCH // MCH):
                        msl = slice(m * MCH, (m + 1) * MCH)
                        for co in range(8):
                            nc.tensor.matmul(
                                ps[:, msl],
                                lhsT=wq_sb[:, co, mt * P : (mt + 1) * P],
                                rhs=xt_sb[:, co, ch * QCH + m * MCH : ch * QCH + (m + 1) * MCH],
                                start=(co == 0),
                                stop=(co == 7),
                            )
                    slo, shi = MT_SLOTS[mt]
                    cp(qk_sb[0:64, slo, sl], ps[0:64, :])
                    cp(qk_sb[64:128, shi, sl], ps[64:128, :])
                for t in range(QCH // P):
                    jt = ch * (QCH // P) + t
                    psv = psV.tile([P, NU, HD], f32, tag="psV")
                    for co in range(8):
                        nc.tensor.matmul(
                            psv,
                            lhsT=xt_sb[:, co, jt * P : (jt + 1) * P],
                            rhs=wv_sb[:, co, :],
                            start=(co == 0),
                            stop=(co == 7),
                        )
                    cp(v1[:, :, jt, 0:HD], psv)

        # ======== scope B: attention, one flat pipeline over (u, jt) ====
        with tc.tile_pool(name="otpool", bufs=1) as otpool:
            OT = otpool.tile([P, 2, nseq], bf16)

            with (
                tc.tile_pool(name="epool", bufs=4) as epool,
                tc.tile_pool(name="obuf", bufs=2) as obuf,
                tc.tile_pool(name="small", bufs=2) as small,
                tc.tile_pool(name="psS", bufs=2, space="PSUM") as psS,
                tc.tile_pool(name="psO", bufs=4, space="PSUM") as psO,
            ):
                psO_units = {}

                def get_psO(u):
                    if u not in psO_units:
                        psO_units[u] = [
                            psO.tile([65, QCH], f32, tag="psO", name=f"psO_{u}_{q}")
                            for q in range(NOB)
                        ]
                    return psO_units[u]

                def emit_S(u, jt):
                    qT_u = qk_sb[:, u, :]
                    kT_u = qk_sb[:, 4 + u, :]
                    ets = []
                    for sw in range(NSW):
                        ps = psS.tile([P, SW], f32, tag="psS")
                        for m in range(SW // MCH):
                            nc.tensor.matmul(
                                ps[:, m * MCH : (m + 1) * MCH],
                                lhsT=kT_u[:, jt * P : (jt + 1) * P],
                                rhs=qT_u[
                                    :, sw * SW + m * MCH : sw * SW + (m + 1) * MCH
                                ],
                                start=True,
                                stop=True,
                            )
                        et = epool.tile([P, SW], bf16, tag="epool")
                        nc.scalar.activation(et, ps, EXP, scale=SCALE)
                        ets.append(et)
                    return ets

                def emit_O(u, jt, ets):
                    tiles = get_psO(u)
                    for sw in range(NSW):
                        for q2 in range(SW // QCH):
                            nc.tensor.matmul(
                                tiles[sw * (SW // QCH) + q2],
                                lhsT=v1[:, u, jt, 0:65],
                                rhs=ets[sw][:, q2 * QCH : (q2 + 1) * QCH],
                                start=(jt == 0),
                                stop=(jt == NJT - 1),
                            )

                def emit_drain(u):
                    # copies release the psO banks; normalize trails on
                    # DVE/gpsimd. For the LAST unit (critical path into proj)
                    # the copies go to ACT, which is idle after the last exp,
                    # leaving DVE free for reciprocal+multiply.
                    last = u == NU - 1
                    pb = 64 * (u % 2)
                    tiles = psO_units.pop(u)
                    o65 = obuf.tile([65, nseq], f32, tag="obuf", name=f"o_{u}")
                    rs = small.tile([1, nseq], f32, tag="rs")
                    recip = small.tile([1, nseq], f32, tag="recip")
                    bcast = small.tile([64, nseq], f32, tag="bcast")
                    cp_op = nc.scalar.copy if last else nc.vector.tensor_copy
                    for q in range(NOB):
                        qsl = slice(q * QCH, (q + 1) * QCH)
                        cp_op(o65[0:64, qsl], tiles[q][0:64, :])
                        # rowsum row to partition 0 first: the custom-DVE
                        # reciprocal mishandles a cross-partition in/out
                        # window on hardware (fine in CoreSim)
                        cp_op(rs[:, qsl], tiles[q][HD : HD + 1, :])
                        nc.vector.reciprocal_approx_fast(recip[:, qsl], rs[:, qsl])
                        nc.gpsimd.partition_broadcast(bcast[:, qsl], recip[:, qsl])
                        nc.vector.tensor_mul(
                            OT[pb : pb + 64, u // 2, qsl],
                            o65[0:64, qsl],
                            bcast[:, qsl],
                        )

                prev = None
                for s in range(NU * NJT):
                    u, jt = divmod(s, NJT)
                    ets = emit_S(u, jt)
                    if prev is not None:
                        emit_O(*prev)
                        if prev[1] == NJT - 1:
                            emit_drain(prev[0])
                    prev = (u, jt, ets)
                emit_O(*prev)
                emit_drain(prev[0])

            # ======== scope C: proj partial out[i, :] ========
            with (
                tc.tile_pool(name="opool", bufs=3) as opool,
                tc.tile_pool(name="psP", bufs=3, space="PSUM") as psP,
            ):
                for it in range(NIT):
                    ps = psP.tile([P, C], f32, tag="psP")
                    # m outer: close each 256-col accumulation group before
                    # opening the next one in the same bank
                    for m in range(C // MCH):
                        for co in range(2):
                            nc.tensor.matmul(
                                ps[:, m * MCH : (m + 1) * MCH],
                                lhsT=OT[:, co, it * P : (it + 1) * P],
                                rhs=wp_sb[:, co, m * MCH : (m + 1) * MCH],
                                start=(co == 0),
                                stop=(co == 1),
                            )
                    ot = opool.tile([P, C], bf16, tag="opool")
                    cp(ot, ps)
                    nc.sync.dma_start(out_d[it * P : (it + 1) * P, :], ot)

    nc.compile()
    return nc


def get_nc(nseq=NSEQ):
    if nseq not in _cache:
        _cache[nseq] = _build(nseq)
    return _cache[nseq]


def make_in_maps(x, w_qkv, w_proj, nseq=NSEQ):
    import ml_dtypes

    bf = ml_dtypes.bfloat16
    x = np.ascontiguousarray(x, dtype=np.float32)
    w_qkv = np.asarray(w_qkv, dtype=np.float32)
    w_proj = np.asarray(w_proj, dtype=np.float32)
    # xT packed [p, co, n] = x[b, n, co*128+p], per batch
    xts = []
    for b in range(B):
        xt = np.ascontiguousarray(
            x[b, :nseq].T.reshape(8, P, nseq).transpose(1, 0, 2).astype(bf)
        )
        xts.append(xt)
    in_maps = []
    for core in range(8):
        b, hg = core // 4, core % 4
        hs = 4 * hg
        # wq [p, co, mt*128+f] = w_qkv[rowbase(mt)+f, co*128+p]
        wq = np.empty((P, 8, 4 * P), np.float32)
        rowbases = [hs * HD, hs * HD + P, C + hs * HD, C + hs * HD + P]
        for mt, r0 in enumerate(rowbases):
            # w_qkv[r0:r0+128, :] -> [f, c]; c = co*128+p
            blk = w_qkv[r0 : r0 + P, :].T.reshape(8, P, P).transpose(1, 0, 2)
            wq[:, :, mt * P : (mt + 1) * P] = blk
        # wv [p, co, u*64+d] = w_qkv[2C+(hs+u)*64+d, co*128+p]
        vblk = w_qkv[2 * C + hs * HD : 2 * C + (hs + 4) * HD, :]  # [256, C]
        wv = vblk.T.reshape(8, P, 4 * HD).transpose(1, 0, 2)
        # wp [p, co, e] = w_proj[e, (hs+2co)*64 + p]
        wp = np.empty((P, 2, C), np.float32)
        for co in range(2):
            c0 = (hs + 2 * co) * HD
            wp[:, co, :] = w_proj[:, c0 : c0 + P].T
        in_maps.append(
            {
                "xT": xts[b],
                "wq": np.ascontiguousarray(wq.astype(bf)),
                "wv": np.ascontiguousarray(wv.astype(bf)),
                "wp": np.ascontiguousarray(wp.astype(bf)),
            }
        )
    return in_maps


def kernel(x, w_qkv, w_proj, b_proj):
    from concourse.bass_utils import run_bass_kernel_spmd

    nc = get_nc()
    in_maps = make_in_maps(x, w_qkv, w_proj)
    res = run_bass_kernel_spmd(nc, in_maps, core_ids=list(range(8)))
    parts = [np.asarray(r["out"], dtype=np.float32) for r in res.results]
    out = np.stack(
        [
            parts[0] + parts[1] + parts[2] + parts[3],
            parts[4] + parts[5] + parts[6] + parts[7],
        ],
        axis=0,
    )
    return (out + np.asarray(b_proj, np.float32)).astype(np.float32)


# revision 11
# speedup vs baseline: 1.0291x; 1.0291x over previous
# Multi-head attention (B=2, N=2048, C=1024, H=16) on 8 trn2 NeuronCores.
#
# Sharding: core = (batch b = core//4, head-group hg = core%4, 4 heads each).
# Each core computes qkv/attention/proj for its 4 heads of its batch and
# returns a partial projection output [N, C] in bf16; the host sums the 4
# partials per batch in f32 and adds b_proj.
#
# All matmul operands are bf16 (PSUM accumulation stays f32); measured
# end-to-end rel_absmax ~7e-3 vs the f32 reference.
#
# Per-core device pipeline:
#   0. Host supplies x already transposed (xT [C, N]) and cast to bf16, so
#      there are no PE transposes at all. Weights packed per-core on host.
#   1. qkT[4*128, N] = Wq/k @ x^T  (lhsT=wq slice, rhs=xT chunk). Matmuls
#      use 256-wide moving chunks: measured slice cost is 112 ns vs 278 ns
#      for 512-wide (the PE SBUF-access latency hides under 256-row
#      streams), so 2x the instructions is still ~20% faster.
#   2. v natural [n, u*64+d] = (xT tile)^T @ wv -> v1[j, u, jt, 0:64],
#      col 64 = ones (rowsum trick).
#   3. Attention as ONE flat 64-step software pipeline over (u, jt): S(s)
#      is issued ahead of O(s-1) across unit boundaries too, so the PE
#      never drains at a head switch. E=exp(S*scale) on ACT -> bf16;
#      O'^T[65, N] += v'^T @ E^T (row 64 = rowsum). psO released by a
#      single [65,512] DVE copy per tile; reciprocal (DVE), chunked
#      partition-broadcast (gpsimd), multiply (DVE) trail behind.
#   4. proj partial out[i,:] per 128-row tile: one [128,1024] psum tile,
#      8 256-wide matmuls, one ACT/DVE copy to bf16, one DMA (issued on
#      sync, which is otherwise idle).
import sys

import numpy as np

if "/opt/trn_rl_repo" not in sys.path:
    sys.path.insert(0, "/opt/trn_rl_repo")

B, NSEQ, C = 2, 2048, 1024
H, HD = 16, 64
P = 128
SCALE = HD**-0.5

_cache = {}


def _build(nseq):
    from contextlib import ExitStack

    import concourse.tile as tile
    from concourse import bacc, mybir

    f32 = mybir.dt.float32
    bf16 = mybir.dt.bfloat16
    EXP = mybir.ActivationFunctionType.Exp

    NJT = nseq // P          # j tiles (keys) per head
    NIT = nseq // P          # i tiles
    QCH = min(512, nseq)     # psum O-tile width
    MCH = 256                # matmul moving-dim chunk
    NCH = nseq // QCH        # number of seq chunks in scope A
    SW = min(1024, nseq)     # S^T psum tile width (2 banks)
    NSW = nseq // SW
    NOB = nseq // QCH        # number of O' psum tiles
    VW = 66                  # v1 row width (65 used: 64 v dims + ones col)
    NU = 4                   # heads per core

    nc = bacc.Bacc("TRN2", target_bir_lowering=False, debug=False, num_devices=8)
    xt_d = nc.dram_tensor("xT", [P, 8, nseq], bf16, kind="ExternalInput")
    wq_d = nc.dram_tensor("wq", [P, 8, 4 * P], bf16, kind="ExternalInput")
    wv_d = nc.dram_tensor("wv", [P, 8, 4 * HD], bf16, kind="ExternalInput")
    wp_d = nc.dram_tensor("wp", [P, 2, C], bf16, kind="ExternalInput")
    out_d = nc.dram_tensor("out", [nseq, C], bf16, kind="ExternalOutput")

    cp_state = [0]

    def cp(out, in_):
        # alternate PSUM->SBUF copies between DVE and ACT
        cp_state[0] ^= 1
        if cp_state[0]:
            nc.vector.tensor_copy(out, in_)
        else:
            nc.scalar.copy(out, in_)

    with tile.TileContext(nc) as tc, ExitStack() as ctx:
        persist = ctx.enter_context(tc.tile_pool(name="persist", bufs=1))

        # ---- input DMAs, priority order: wq by mt, xT chunk 0, rest ----
        wq_sb = persist.tile([P, 8, 4 * P], bf16)
        xt_sb = persist.tile([P, 8, nseq], bf16)
        wv_sb = persist.tile([P, 8, 4 * HD], bf16)
        wp_sb = persist.tile([P, 2, C], bf16)
        nc.sync.dma_start(wq_sb[:, :, 0:2 * P], wq_d[:, :, 0:2 * P])
        nc.scalar.dma_start(wq_sb[:, :, 2 * P:4 * P], wq_d[:, :, 2 * P:4 * P])
        dma_engines = [nc.sync, nc.scalar, nc.gpsimd]
        for co in range(8):  # first seq chunk, highest priority
            dma_engines[co % 3].dma_start(
                xt_sb[:, co, 0:QCH], xt_d[:, co, 0:QCH]
            )
        nc.gpsimd.dma_start(wv_sb, wv_d.ap())
        for co in range(8):  # rest of the sequence
            dma_engines[co % 3].dma_start(
                xt_sb[:, co, QCH:nseq], xt_d[:, co, QCH:nseq]
            )
        nc.gpsimd.dma_start(wp_sb, wp_d.ap())

        # q^T/k^T per head, zero-padded to full 128 partitions.
        # slot u = q of head u; slot 4+u = k of head u.
        qk_sb = persist.tile([P, 8, nseq], bf16)
        # v' natural [j_part, u, jt, 0:64]=v, col 64 = ones.
        v1 = persist.tile([P, NU, NJT, VW], bf16)
        for slot in range(8):
            zpb = 64 if slot % 2 == 0 else 0
            nc.vector.memset(qk_sb[zpb : zpb + 64, slot, :], 0.0)
        nc.vector.memset(v1[:, :, :, HD : HD + 1], 1.0)

        # prime the ACT exp table early so head 0 doesn't stall on it
        ones_f32 = persist.tile([P, 1], f32)
        nc.vector.memset(ones_f32, 1.0)
        prime = persist.tile([P, 1], f32)
        nc.scalar.activation(prime, ones_f32, EXP, scale=0.0)

        # ======== scope A: qk matmuls + v-natural build ========
        # mt 0 = q heads (0,1), mt 1 = q heads (2,3), mt 2 = k (0,1),
        # mt 3 = k (2,3). psQ partitions 0:64 = first head of pair.
        MT_SLOTS = [(0, 1), (2, 3), (4, 5), (6, 7)]
        with (
            tc.tile_pool(name="psQ", bufs=3, space="PSUM") as psQ,
            tc.tile_pool(name="psV", bufs=3, space="PSUM") as psV,
        ):
            for ch in range(NCH):
                sl = slice(ch * QCH, (ch + 1) * QCH)
                for mt in range(4):
                    ps = psQ.tile([P, QCH], f32, tag="psQ")
                    # m outer: one open accumulation group per psum bank at
                    # a time (interleaved open groups corrupt accumulation)
                    for m in range(QCH // MCH):
                        msl = slice(m * MCH, (m + 1) * MCH)
                        for co in range(8):
                            nc.tensor.matmul(
                                ps[:, msl],
                                lhsT=wq_sb[:, co, mt * P : (mt + 1) * P],
                                rhs=xt_sb[:, co, ch * QCH + m * MCH : ch * QCH + (m + 1) * MCH],
                                start=(co == 0),
                                stop=(co == 7),
                            )
                    slo, shi = MT_SLOTS[mt]
                    cp(qk_sb[0:64, slo, sl], ps[0:64, :])
                    cp(qk_sb[64:128, shi, sl], ps[64:128, :])
                for t in range(QCH // P):
                    jt = ch * (QCH // P) + t
                    psv = psV.tile([P, NU, HD], f32, tag="psV")
                    for co in range(8):
                        nc.tensor.matmul(
                            psv,
                            lhsT=xt_sb[:, co, jt * P : (jt + 1) * P],
                            rhs=wv_sb[:, co, :],
                            start=(co == 0),
                            stop=(co == 7),
                        )
                    cp(v1[:, :, jt, 0:HD], psv)

        # ======== scope B: attention, one flat pipeline over (u, jt) ====
        with tc.tile_pool(name="otpool", bufs=1) as otpool:
            OT = otpool.tile([P, 2, nseq], bf16)

            with (
                tc.tile_pool(name="epool", bufs=4) as epool,
                tc.tile_pool(name="obuf", bufs=2) as obuf,
                tc.tile_pool(name="small", bufs=2) as small,
                tc.tile_pool(name="psS", bufs=2, space="PSUM") as psS,
                tc.tile_pool(name="psO", bufs=4, space="PSUM") as psO,
            ):
                psO_units = {}

                def get_psO(u):
                    if u not in psO_units:
                        psO_units[u] = [
                            psO.tile([65, QCH], f32, tag="psO", name=f"psO_{u}_{q}")
                            for q in range(NOB)
                        ]
                    return psO_units[u]

                def emit_S(u, jt):
                    qT_u = qk_sb[:, u, :]
                    kT_u = qk_sb[:, 4 + u, :]
                    ets = []
                    for sw in range(NSW):
                        ps = psS.tile([P, SW], f32, tag="psS")
                        for m in range(SW // MCH):
                            nc.tensor.matmul(
                                ps[:, m * MCH : (m + 1) * MCH],
                                lhsT=kT_u[:, jt * P : (jt + 1) * P],
                                rhs=qT_u[
                                    :, sw * SW + m * MCH : sw * SW + (m + 1) * MCH
                                ],
                                start=True,
                                stop=True,
                            )
                        et = epool.tile([P, SW], bf16, tag="epool")
                        nc.scalar.activation(et, ps, EXP, scale=SCALE)
                        ets.append(et)
                    return ets

                def emit_O(u, jt, ets):
                    tiles = get_psO(u)
                    for sw in range(NSW):
                        for q2 in range(SW // QCH):
                            nc.tensor.matmul(
                                tiles[sw * (SW // QCH) + q2],
                                lhsT=v1[:, u, jt, 0:65],
                                rhs=ets[sw][:, q2 * QCH : (q2 + 1) * QCH],
                                start=(jt == 0),
                                stop=(jt == NJT - 1),
                            )

                def emit_drain(u):
                    # copies release the psO banks; normalize trails on
                    # DVE/gpsimd. For the LAST unit (critical path into proj)
                    # the copies go to ACT, which is idle after the last exp,
                    # leaving DVE free for reciprocal+multiply.
                    last = u == NU - 1
                    pb = 64 * (u % 2)
                    tiles = psO_units.pop(u)
                    o65 = obuf.tile([65, nseq], f32, tag="obuf", name=f"o_{u}")
                    rs = small.tile([1, nseq], f32, tag="rs")
                    recip = small.tile([1, nseq], f32, tag="recip")
                    bcast = small.tile([64, nseq], f32, tag="bcast")
                    cp_op = nc.scalar.copy if last else nc.vector.tensor_copy
                    for q in range(NOB):
                        # single [65,512] copy per tile releases the psO bank
                        qsl = slice(q * QCH, (q + 1) * QCH)
                        cp_op(o65[:, qsl], tiles[q])
                    for q in range(NOB):
                        qsl = slice(q * QCH, (q + 1) * QCH)
                        # rowsum row to partition 0 first: the custom-DVE
                        # reciprocal mishandles a cross-partition in/out
                        # window on hardware (fine in CoreSim)
                        cp_op(rs[:, qsl], o65[64:65, qsl])
                        nc.vector.reciprocal_approx_fast(recip[:, qsl], rs[:, qsl])
                        nc.gpsimd.partition_broadcast(bcast[:, qsl], recip[:, qsl])
                        nc.vector.tensor_mul(
                            OT[pb : pb + 64, u // 2, qsl],
                            o65[0:64, qsl],
                            bcast[:, qsl],
                        )

                prev = None
                for s in range(NU * NJT):
                    u, jt = divmod(s, NJT)
                    ets = emit_S(u, jt)
                    if prev is not None:
                        emit_O(*prev)
                        if prev[1] == NJT - 1:
                            emit_drain(prev[0])
                    prev = (u, jt, ets)
                emit_O(*prev)
                emit_drain(prev[0])

            # ======== scope C: proj partial out[i, :] ========
            with (
                tc.tile_pool(name="opool", bufs=3) as opool,
                tc.tile_pool(name="psP", bufs=3, space="PSUM") as psP,
            ):
                for it in range(NIT):
                    ps = psP.tile([P, C], f32, tag="psP")
                    # m outer: close each 256-col accumulation group before
                    # opening the next one in the same bank
                    for m in range(C // MCH):
                        for co in range(2):
                            nc.tensor.matmul(
                                ps[:, m * MCH : (m + 1) * MCH],
                                lhsT=OT[:, co, it * P : (it + 1) * P],
                                rhs=wp_sb[:, co, m * MCH : (m + 1) * MCH],
                                start=(co == 0),
                                stop=(co == 1),
                            )
                    ot = opool.tile([P, C], bf16, tag="opool")
                    cp(ot, ps)
                    nc.sync.dma_start(out_d[it * P : (it + 1) * P, :], ot)

    nc.compile()
    return nc


def get_nc(nseq=NSEQ):
    if nseq not in _cache:
        _cache[nseq] = _build(nseq)
    return _cache[nseq]


def make_in_maps(x, w_qkv, w_proj, nseq=NSEQ):
    import ml_dtypes

    bf = ml_dtypes.bfloat16
    x = np.ascontiguousarray(x, dtype=np.float32)
    w_qkv = np.asarray(w_qkv, dtype=np.float32)
    w_proj = np.asarray(w_proj, dtype=np.float32)
    # xT packed [p, co, n] = x[b, n, co*128+p], per batch
    xts = []
    for b in range(B):
        xt = np.ascontiguousarray(
            x[b, :nseq].T.reshape(8, P, nseq).transpose(1, 0, 2).astype(bf)
        )
        xts.append(xt)
    in_maps = []
    for core in range(8):
        b, hg = core // 4, core % 4
        hs = 4 * hg
        # wq [p, co, mt*128+f] = w_qkv[rowbase(mt)+f, co*128+p]
        wq = np.empty((P, 8, 4 * P), np.float32)
        rowbases = [hs * HD, hs * HD + P, C + hs * HD, C + hs * HD + P]
        for mt, r0 in enumerate(rowbases):
            # w_qkv[r0:r0+128, :] -> [f, c]; c = co*128+p
            blk = w_qkv[r0 : r0 + P, :].T.reshape(8, P, P).transpose(1, 0, 2)
            wq[:, :, mt * P : (mt + 1) * P] = blk
        # wv [p, co, u*64+d] = w_qkv[2C+(hs+u)*64+d, co*128+p]
        vblk = w_qkv[2 * C + hs * HD : 2 * C + (hs + 4) * HD, :]  # [256, C]
        wv = vblk.T.reshape(8, P, 4 * HD).transpose(1, 0, 2)
        # wp [p, co, e] = w_proj[e, (hs+2co)*64 + p]
        wp = np.empty((P, 2, C), np.float32)
        for co in range(2):
            c0 = (hs + 2 * co) * HD
            wp[:, co, :] = w_proj[:, c0 : c0 + P].T
        in_maps.append(
            {
                "xT": xts[b],
                "wq": np.ascontiguousarray(wq.astype(bf)),
                "wv": np.ascontiguousarray(wv.astype(bf)),
                "wp": np.ascontiguousarray(wp.astype(bf)),
            }
        )
    return in_maps


def kernel(x, w_qkv, w_proj, b_proj):
    from concourse.bass_utils import run_bass_kernel_spmd

    nc = get_nc()
    in_maps = make_in_maps(x, w_qkv, w_proj)
    res = run_bass_kernel_spmd(nc, in_maps, core_ids=list(range(8)))
    parts = [np.asarray(r["out"], dtype=np.float32) for r in res.results]
    out = np.stack(
        [
            parts[0] + parts[1] + parts[2] + parts[3],
            parts[4] + parts[5] + parts[6] + parts[7],
        ],
        axis=0,
    )
    return (out + np.asarray(b_proj, np.float32)).astype(np.float32)


# revision 16
# speedup vs baseline: 1.0342x; 1.0049x over previous
# Multi-head attention (B=2, N=2048, C=1024, H=16) on 8 trn2 NeuronCores.
#
# Sharding: core = (batch b = core//4, head-group hg = core%4, 4 heads each).
# Each core computes qkv/attention/proj for its 4 heads of its batch and
# returns a partial projection output [N, C] in bf16; the host sums the 4
# partials per batch in f32 and adds b_proj.
#
# All matmul operands are bf16 (PSUM accumulation stays f32); measured
# end-to-end rel_absmax ~7e-3 vs the f32 reference.
#
# Per-core device pipeline:
#   0. Host supplies x already transposed (xT [C, N]) and cast to bf16, so
#      there are no PE transposes at all. Weights packed per-core on host.
#   1. qkT[4*128, N] = Wq/k @ x^T  (lhsT=wq slice, rhs=xT chunk). Matmuls
#      use 256-wide moving chunks: measured slice cost is 112 ns vs 278 ns
#      for 512-wide (the PE SBUF-access latency hides under 256-row
#      streams), so 2x the instructions is still ~20% faster.
#   2. v natural [n, u*64+d] = (xT tile)^T @ wv -> v1[j, u, jt, 0:64],
#      col 64 = ones (rowsum trick).
#   3. Attention as ONE flat 64-step software pipeline over (u, jt): S(s)
#      is issued ahead of O(s-1) across unit boundaries too, so the PE
#      never drains at a head switch. E=exp(S*scale) on ACT -> bf16;
#      O'^T[65, N] += v'^T @ E^T (row 64 = rowsum). psO released by a
#      single [65,512] DVE copy per tile; reciprocal (DVE), chunked
#      partition-broadcast (gpsimd), multiply (DVE) trail behind.
#   4. proj partial out[i,:] per 128-row tile: one [128,1024] psum tile,
#      8 256-wide matmuls, one ACT/DVE copy to bf16, one DMA (issued on
#      sync, which is otherwise idle).
import sys

import numpy as np

if "/opt/trn_rl_repo" not in sys.path:
    sys.path.insert(0, "/opt/trn_rl_repo")

B, NSEQ, C = 2, 2048, 1024
H, HD = 16, 64
P = 128
SCALE = HD**-0.5

_cache = {}


def _build(nseq):
    from contextlib import ExitStack

    import concourse.tile as tile
    from concourse import bacc, mybir

    f32 = mybir.dt.float32
    bf16 = mybir.dt.bfloat16
    EXP = mybir.ActivationFunctionType.Exp

    NJT = nseq // P          # j tiles (keys) per head
    NIT = nseq // P          # i tiles
    QCH = min(512, nseq)     # psum O-tile width
    MCH = 256                # matmul moving-dim chunk
    NCH = nseq // QCH        # number of seq chunks in scope A
    SW = min(1024, nseq)     # S^T psum tile width (2 banks)
    NSW = nseq // SW
    NOB = nseq // QCH        # number of O' psum tiles
    VW = 66                  # v1 row width (65 used: 64 v dims + ones col)
    NU = 4                   # heads per core

    nc = bacc.Bacc("TRN2", target_bir_lowering=False, debug=False, num_devices=8)
    xt_d = nc.dram_tensor("xT", [P, 8, nseq], bf16, kind="ExternalInput")
    wq_d = nc.dram_tensor("wq", [P, 8, 4 * P], bf16, kind="ExternalInput")
    wv_d = nc.dram_tensor("wv", [P, 8, 4 * HD], bf16, kind="ExternalInput")
    wp_d = nc.dram_tensor("wp", [P, 2, C], bf16, kind="ExternalInput")
    out_d = nc.dram_tensor("out", [nseq, C], bf16, kind="ExternalOutput")

    cp_state = [0]

    def cp(out, in_):
        # alternate PSUM->SBUF copies between DVE and ACT
        cp_state[0] ^= 1
        if cp_state[0]:
            nc.vector.tensor_copy(out, in_)
        else:
            nc.scalar.copy(out, in_)

    with tile.TileContext(nc) as tc, ExitStack() as ctx:
        persist = ctx.enter_context(tc.tile_pool(name="persist", bufs=1))

        # ---- input DMAs, priority order: wq by mt, xT chunk 0, rest ----
        wq_sb = persist.tile([P, 8, 4 * P], bf16)
        xt_sb = persist.tile([P, 8, nseq], bf16)
        wv_sb = persist.tile([P, 8, 4 * HD], bf16)
        wp_sb = persist.tile([P, 2, C], bf16)
        nc.sync.dma_start(wq_sb[:, :, 0:2 * P], wq_d[:, :, 0:2 * P])
        nc.scalar.dma_start(wq_sb[:, :, 2 * P:4 * P], wq_d[:, :, 2 * P:4 * P])
        dma_engines = [nc.sync, nc.scalar, nc.gpsimd]
        for m in range(QCH // MCH):  # first seq chunk in 256-col quarters:
            for co in range(8):  # the first qk matmul chain needs only 0:256
                dma_engines[co % 3].dma_start(
                    xt_sb[:, co, m * MCH : (m + 1) * MCH],
                    xt_d[:, co, m * MCH : (m + 1) * MCH],
                )
        nc.gpsimd.dma_start(wv_sb, wv_d.ap())
        for co in range(8):  # rest of the sequence
            dma_engines[co % 3].dma_start(
                xt_sb[:, co, QCH:nseq], xt_d[:, co, QCH:nseq]
            )
        nc.gpsimd.dma_start(wp_sb, wp_d.ap())

        # q^T/k^T per head, zero-padded to full 128 partitions.
        # slot u = q of head u; slot 4+u = k of head u.
        qk_sb = persist.tile([P, 8, nseq], bf16)
        # v' natural [j_part, u, jt, 0:64]=v, col 64 = ones.
        v1 = persist.tile([P, NU, NJT, VW], bf16)
        for slot in range(8):
            zpb = 64 if slot % 2 == 0 else 0
            nc.vector.memset(qk_sb[zpb : zpb + 64, slot, :], 0.0)
        nc.vector.memset(v1[:, :, :, HD : HD + 1], 1.0)

        # prime the ACT exp table early so head 0 doesn't stall on it
        ones_f32 = persist.tile([P, 1], f32)
        nc.vector.memset(ones_f32, 1.0)
        prime = persist.tile([P, 1], f32)
        nc.scalar.activation(prime, ones_f32, EXP, scale=0.0)

        # ======== scope A: qk matmuls + v-natural build ========
        # mt 0 = q heads (0,1), mt 1 = q heads (2,3), mt 2 = k (0,1),
        # mt 3 = k (2,3). psQ partitions 0:64 = first head of pair.
        MT_SLOTS = [(0, 1), (2, 3), (4, 5), (6, 7)]
        with (
            tc.tile_pool(name="psQ", bufs=2, space="PSUM") as psQ,
            tc.tile_pool(name="psV", bufs=2, space="PSUM") as psV,
        ):
            for ch in range(NCH):
                sl = slice(ch * QCH, (ch + 1) * QCH)
                for mt in (0, 2, 1, 3):  # q01, k01 first: head 0's S only
                    # waits on the q01/k01 copies at the attention handoff
                    ps = psQ.tile([P, QCH], f32, tag="psQ")
                    # m outer: one open accumulation group per psum bank at
                    # a time (interleaved open groups corrupt accumulation)
                    for m in range(QCH // MCH):
                        msl = slice(m * MCH, (m + 1) * MCH)
                        for co in range(8):
                            nc.tensor.matmul(
                                ps[:, msl],
                                lhsT=wq_sb[:, co, mt * P : (mt + 1) * P],
                                rhs=xt_sb[:, co, ch * QCH + m * MCH : ch * QCH + (m + 1) * MCH],
                                start=(co == 0),
                                stop=(co == 7),
                            )
                    slo, shi = MT_SLOTS[mt]
                    cp(qk_sb[0:64, slo, sl], ps[0:64, :])
                    cp(qk_sb[64:128, shi, sl], ps[64:128, :])
                for t in range(QCH // P):
                    jt = ch * (QCH // P) + t
                    psv = psV.tile([P, NU, HD], f32, tag="psV")
                    for co in range(8):
                        nc.tensor.matmul(
                            psv,
                            lhsT=xt_sb[:, co, jt * P : (jt + 1) * P],
                            rhs=wv_sb[:, co, :],
                            start=(co == 0),
                            stop=(co == 7),
                        )
                    cp(v1[:, :, jt, 0:HD], psv)

        # ======== scope B: attention, one flat pipeline over (u, jt) ====
        with tc.tile_pool(name="otpool", bufs=1) as otpool:
            OT = otpool.tile([P, 2, nseq], bf16)

            with (
                tc.tile_pool(name="epool", bufs=4) as epool,
                tc.tile_pool(name="obuf", bufs=2) as obuf,
                tc.tile_pool(name="small", bufs=2) as small,
                tc.tile_pool(name="psS", bufs=2, space="PSUM") as psS,
                tc.tile_pool(name="psO", bufs=4, space="PSUM") as psO,
            ):
                psO_units = {}

                def get_psO(u):
                    if u not in psO_units:
                        psO_units[u] = [
                            psO.tile([65, QCH], f32, tag="psO", name=f"psO_{u}_{q}")
                            for q in range(NOB)
                        ]
                    return psO_units[u]

                def emit_S(u, jt):
                    qT_u = qk_sb[:, u, :]
                    kT_u = qk_sb[:, 4 + u, :]
                    ets = []
                    for sw in range(NSW):
                        ps = psS.tile([P, SW], f32, tag="psS")
                        for m in range(SW // MCH):
                            nc.tensor.matmul(
                                ps[:, m * MCH : (m + 1) * MCH],
                                lhsT=kT_u[:, jt * P : (jt + 1) * P],
                                rhs=qT_u[
                                    :, sw * SW + m * MCH : sw * SW + (m + 1) * MCH
                                ],
                                start=True,
                                stop=True,
                            )
                        et = epool.tile([P, SW], bf16, tag="epool")
                        nc.scalar.activation(et, ps, EXP, scale=SCALE)
                        ets.append(et)
                    return ets

                def emit_O(u, jt, ets):
                    tiles = get_psO(u)
                    for sw in range(NSW):
                        for q2 in range(SW // QCH):
                            nc.tensor.matmul(
                                tiles[sw * (SW // QCH) + q2],
                                lhsT=v1[:, u, jt, 0:65],
                                rhs=ets[sw][:, q2 * QCH : (q2 + 1) * QCH],
                                start=(jt == 0),
                                stop=(jt == NJT - 1),
                            )

                def emit_drain(u):
                    # copies release the psO banks; normalize trails on
                    # DVE/gpsimd. For the LAST unit (critical path into proj)
                    # the copies go to ACT, which is idle after the last exp,
                    # leaving DVE free for reciprocal+multiply.
                    pb = 64 * (u % 2)
                    tiles = psO_units.pop(u)
                    o65 = obuf.tile([65, nseq], f32, tag="obuf", name=f"o_{u}")
                    rs = small.tile([1, nseq], f32, tag="rs")
                    recip = small.tile([1, nseq], f32, tag="recip")
                    bcast = small.tile([64, nseq], f32, tag="bcast")
                    cp_op = nc.vector.tensor_copy
                    for q in range(NOB):
                        # single [65,512] copy per tile releases the psO bank
                        qsl = slice(q * QCH, (q + 1) * QCH)
                        cp_op(o65[:, qsl], tiles[q])
                    for q in range(NOB):
                        qsl = slice(q * QCH, (q + 1) * QCH)
                        # rowsum row to partition 0 first: the custom-DVE
                        # reciprocal mishandles a cross-partition in/out
                        # window on hardware (fine in CoreSim)
                        cp_op(rs[:, qsl], o65[64:65, qsl])
                        nc.vector.reciprocal_approx_fast(recip[:, qsl], rs[:, qsl])
                        nc.gpsimd.partition_broadcast(bcast[:, qsl], recip[:, qsl])
                        nc.vector.tensor_mul(
                            OT[pb : pb + 64, u // 2, qsl],
                            o65[0:64, qsl],
                            bcast[:, qsl],
                        )

                prev = None
                for s in range(NU * NJT):
                    u, jt = divmod(s, NJT)
                    ets = emit_S(u, jt)
                    if prev is not None:
                        emit_O(*prev)
                        if prev[1] == NJT - 1:
                            emit_drain(prev[0])
                    prev = (u, jt, ets)
                emit_O(*prev)
                emit_drain(prev[0])

            # ======== scope C: proj partial out[i, :] ========
            with (
                tc.tile_pool(name="opool", bufs=3) as opool,
                tc.tile_pool(name="psP", bufs=3, space="PSUM") as psP,
            ):
                for it in range(NIT):
                    ps = psP.tile([P, C], f32, tag="psP")
                    # m outer: close each 256-col accumulation group before
                    # opening the next one in the same bank
                    for m in range(C // MCH):
                        for co in range(2):
                            nc.tensor.matmul(
                                ps[:, m * MCH : (m + 1) * MCH],
                                lhsT=OT[:, co, it * P : (it + 1) * P],
                                rhs=wp_sb[:, co, m * MCH : (m + 1) * MCH],
                                start=(co == 0),
                                stop=(co == 1),
                            )
                    ot = opool.tile([P, C], bf16, tag="opool")
                    cp(ot, ps)
                    nc.sync.dma_start(out_d[it * P : (it + 1) * P, :], ot)

    nc.compile()
    return nc


def get_nc(nseq=NSEQ):
    if nseq not in _cache:
        _cache[nseq] = _build(nseq)
    return _cache[nseq]


def make_in_maps(x, w_qkv, w_proj, nseq=NSEQ):
    import ml_dtypes

    bf = ml_dtypes.bfloat16
    x = np.ascontiguousarray(x, dtype=np.float32)
    w_qkv = np.asarray(w_qkv, dtype=np.float32)
    w_proj = np.asarray(w_proj, dtype=np.float32)
    # xT packed [p, co, n] = x[b, n, co*128+p], per batch
    xts = []
    for b in range(B):
        xt = np.ascontiguousarray(
            x[b, :nseq].T.reshape(8, P, nseq).transpose(1, 0, 2).astype(bf)
        )
        xts.append(xt)
    in_maps = []
    for core in range(8):
        b, hg = core // 4, core % 4
        hs = 4 * hg
        # wq [p, co, mt*128+f] = w_qkv[rowbase(mt)+f, co*128+p]
        wq = np.empty((P, 8, 4 * P), np.float32)
        rowbases = [hs * HD, hs * HD + P, C + hs * HD, C + hs * HD + P]
        for mt, r0 in enumerate(rowbases):
            # w_qkv[r0:r0+128, :] -> [f, c]; c = co*128+p
            blk = w_qkv[r0 : r0 + P, :].T.reshape(8, P, P).transpose(1, 0, 2)
            wq[:, :, mt * P : (mt + 1) * P] = blk
        # wv [p, co, u*64+d] = w_qkv[2C+(hs+u)*64+d, co*128+p]
        vblk = w_qkv[2 * C + hs * HD : 2 * C + (hs + 4) * HD, :]  # [256, C]
        wv = vblk.T.reshape(8, P, 4 * HD).transpose(1, 0, 2)
        # wp [p, co, e] = w_proj[e, (hs+2co)*64 + p]
        wp = np.empty((P, 2, C), np.float32)
        for co in range(2):
            c0 = (hs + 2 * co) * HD
            wp[:, co, :] = w_proj[:, c0 : c0 + P].T
        in_maps.append(
            {
                "xT": xts[b],
                "wq": np.ascontiguousarray(wq.astype(bf)),
                "wv": np.ascontiguousarray(wv.astype(bf)),
                "wp": np.ascontiguousarray(wp.astype(bf)),
            }
        )
    return in_maps


def kernel(x, w_qkv, w_proj, b_proj):
    from concourse.bass_utils import run_bass_kernel_spmd

    nc = get_nc()
    in_maps = make_in_maps(x, w_qkv, w_proj)
    res = run_bass_kernel_spmd(nc, in_maps, core_ids=list(range(8)))
    parts = [np.asarray(r["out"], dtype=np.float32) for r in res.results]
    out = np.stack(
        [
            parts[0] + parts[1] + parts[2] + parts[3],
            parts[4] + parts[5] + parts[6] + parts[7],
        ],
        axis=0,
    )
    return (out + np.asarray(b_proj, np.float32)).astype(np.float32)


# revision 19
# speedup vs baseline: 1.0478x; 1.0132x over previous
# Multi-head attention (B=2, N=2048, C=1024, H=16) on 8 trn2 NeuronCores.
#
# Sharding: core = (batch b = core//4, head-group hg = core%4, 4 heads each).
# Each core computes qkv/attention/proj for its 4 heads of its batch and
# returns a partial projection output [N, C] in bf16; the host sums the 4
# partials per batch in f32 and adds b_proj.
#
# All matmul operands are bf16 (PSUM accumulation stays f32); measured
# end-to-end rel_absmax ~7e-3 vs the f32 reference.
#
# Per-core device pipeline:
#   0. Host supplies x already transposed (xT [C, N]) and cast to bf16, so
#      there are no PE transposes at all. Weights packed per-core on host.
#   1. qkT[4*128, N] = Wq/k @ x^T  (lhsT=wq slice, rhs=xT chunk). Matmuls
#      use 256-wide moving chunks: measured slice cost is 112 ns vs 278 ns
#      for 512-wide (the PE SBUF-access latency hides under 256-row
#      streams), so 2x the instructions is still ~20% faster.
#   2. v natural [n, u*64+d] = (xT tile)^T @ wv -> v1[j, u, jt, 0:64],
#      col 64 = ones (rowsum trick).
#   3. Attention as ONE flat 64-step software pipeline over (u, jt): S(s)
#      is issued ahead of O(s-1) across unit boundaries too, so the PE
#      never drains at a head switch. E=exp(S*scale) on ACT -> bf16;
#      O'^T[65, N] += v'^T @ E^T (row 64 = rowsum). psO released by a
#      single [65,512] DVE copy per tile; reciprocal (DVE), chunked
#      partition-broadcast (gpsimd), multiply (DVE) trail behind.
#   4. proj partial out[i,:] per 128-row tile: one [128,1024] psum tile,
#      8 256-wide matmuls, one ACT/DVE copy to bf16, one DMA (issued on
#      sync, which is otherwise idle).
import sys

import numpy as np

if "/opt/trn_rl_repo" not in sys.path:
    sys.path.insert(0, "/opt/trn_rl_repo")

B, NSEQ, C = 2, 2048, 1024
H, HD = 16, 64
P = 128
SCALE = HD**-0.5

_cache = {}


def _build(nseq):
    from contextlib import ExitStack

    import concourse.tile as tile
    from concourse import bacc, mybir

    f32 = mybir.dt.float32
    bf16 = mybir.dt.bfloat16
    EXP = mybir.ActivationFunctionType.Exp

    NJT = nseq // P          # j tiles (keys) per head
    NIT = nseq // P          # i tiles
    QCH = min(512, nseq)     # psum O-tile width
    MCH = 256                # matmul moving-dim chunk
    NCH = nseq // QCH        # number of seq chunks in scope A
    SW = min(1024, nseq)     # S^T psum tile width (2 banks)
    NSW = nseq // SW
    NOB = nseq // QCH        # number of O' psum tiles
    VW = 66                  # v1 row width (65 used: 64 v dims + ones col)
    NU = 4                   # heads per core

    nc = bacc.Bacc("TRN2", target_bir_lowering=False, debug=False, num_devices=8)
    xt_d = nc.dram_tensor("xT", [P, 8, nseq], bf16, kind="ExternalInput")
    wq_d = nc.dram_tensor("wq", [P, 8, 4 * P], bf16, kind="ExternalInput")
    wv_d = nc.dram_tensor("wv", [P, 8, 4 * HD], bf16, kind="ExternalInput")
    wp_d = nc.dram_tensor("wp", [P, 2, C], bf16, kind="ExternalInput")
    out_d = nc.dram_tensor("out", [nseq, C], bf16, kind="ExternalOutput")

    cp_state = [0]

    def cp(out, in_):
        # alternate PSUM->SBUF copies between DVE and ACT
        cp_state[0] ^= 1
        if cp_state[0]:
            nc.vector.tensor_copy(out, in_)
        else:
            nc.scalar.copy(out, in_)

    with tile.TileContext(nc) as tc, ExitStack() as ctx:
        persist = ctx.enter_context(tc.tile_pool(name="persist", bufs=1))

        # ---- input DMAs, priority order: wq by mt, xT chunk 0, rest ----
        wq_sb = persist.tile([P, 8, 4 * P], bf16)
        xt_sb = persist.tile([P, 8, nseq], bf16)
        wv_sb = persist.tile([P, 8, 4 * HD], bf16)
        wp_sb = persist.tile([P, 2, C], bf16)
        nc.sync.dma_start(wq_sb[:, :, 0:2 * P], wq_d[:, :, 0:2 * P])
        nc.scalar.dma_start(wq_sb[:, :, 2 * P:4 * P], wq_d[:, :, 2 * P:4 * P])
        dma_engines = [nc.sync, nc.scalar, nc.gpsimd]
        for m in range(QCH // MCH):  # first seq chunk in 256-col quarters:
            for co in range(8):  # the first qk matmul chain needs only 0:256
                dma_engines[co % 3].dma_start(
                    xt_sb[:, co, m * MCH : (m + 1) * MCH],
                    xt_d[:, co, m * MCH : (m + 1) * MCH],
                )
        nc.gpsimd.dma_start(wv_sb, wv_d.ap())
        for co in range(8):  # rest of the sequence
            dma_engines[co % 3].dma_start(
                xt_sb[:, co, QCH:nseq], xt_d[:, co, QCH:nseq]
            )
        nc.gpsimd.dma_start(wp_sb, wp_d.ap())

        # q^T/k^T per head, zero-padded to full 128 partitions.
        # slot u = q of head u; slot 4+u = k of head u.
        qk_sb = persist.tile([P, 8, nseq], bf16)
        # v' natural [j_part, u, jt, 0:64]=v, col 64 = ones.
        v1 = persist.tile([P, NU, NJT, VW], bf16)
        for slot in range(8):
            zpb = 64 if slot % 2 == 0 else 0
            nc.vector.memset(qk_sb[zpb : zpb + 64, slot, :], 0.0)
        nc.vector.memset(v1[:, :, :, HD : HD + 1], 1.0)

        # prime the ACT exp table early so head 0 doesn't stall on it
        ones_f32 = persist.tile([P, 1], f32)
        nc.vector.memset(ones_f32, 1.0)
        prime = persist.tile([P, 1], f32)
        nc.scalar.activation(prime, ones_f32, EXP, scale=0.0)

        # ======== scope A: qk matmuls + v-natural build ========
        # mt 0 = q heads (0,1), mt 1 = q heads (2,3), mt 2 = k (0,1),
        # mt 3 = k (2,3). psQ partitions 0:64 = first head of pair.
        MT_SLOTS = [(0, 1), (2, 3), (4, 5), (6, 7)]
        with (
            tc.tile_pool(name="psQ", bufs=2, space="PSUM") as psQ,
            tc.tile_pool(name="psV", bufs=2, space="PSUM") as psV,
        ):
            for ch in range(NCH):
                sl = slice(ch * QCH, (ch + 1) * QCH)
                for mt in range(4):
                    ps = psQ.tile([P, QCH], f32, tag="psQ")
                    # m outer: one open accumulation group per psum bank at
                    # a time (interleaved open groups corrupt accumulation)
                    for m in range(QCH // MCH):
                        msl = slice(m * MCH, (m + 1) * MCH)
                        for co in range(8):
                            nc.tensor.matmul(
                                ps[:, msl],
                                lhsT=wq_sb[:, co, mt * P : (mt + 1) * P],
                                rhs=xt_sb[:, co, ch * QCH + m * MCH : ch * QCH + (m + 1) * MCH],
                                start=(co == 0),
                                stop=(co == 7),
                            )
                    slo, shi = MT_SLOTS[mt]
                    cp(qk_sb[0:64, slo, sl], ps[0:64, :])
                    cp(qk_sb[64:128, shi, sl], ps[64:128, :])
                for t in range(QCH // P):
                    jt = ch * (QCH // P) + t
                    psv = psV.tile([P, NU, HD], f32, tag="psV")
                    for co in range(8):
                        nc.tensor.matmul(
                            psv,
                            lhsT=xt_sb[:, co, jt * P : (jt + 1) * P],
                            rhs=wv_sb[:, co, :],
                            start=(co == 0),
                            stop=(co == 7),
                        )
                    cp(v1[:, :, jt, 0:HD], psv)

        # ======== scope B: attention, one flat pipeline over (u, jt) ====
        with tc.tile_pool(name="otpool", bufs=1) as otpool:
            OT = otpool.tile([P, 2, nseq], bf16)

            with (
                tc.tile_pool(name="epool", bufs=4) as epool,
                tc.tile_pool(name="obuf", bufs=2) as obuf,
                tc.tile_pool(name="small", bufs=2) as small,
                tc.tile_pool(name="psS", bufs=2, space="PSUM") as psS,
                tc.tile_pool(name="psO", bufs=4, space="PSUM") as psO,
            ):
                psO_units = {}

                def get_psO(u):
                    if u not in psO_units:
                        psO_units[u] = [
                            psO.tile([65, QCH], f32, tag="psO", name=f"psO_{u}_{q}")
                            for q in range(NOB)
                        ]
                    return psO_units[u]

                def emit_S(u, jt):
                    qT_u = qk_sb[:, u, :]
                    kT_u = qk_sb[:, 4 + u, :]
                    ets = []
                    for sw in range(NSW):
                        ps = psS.tile([P, SW], f32, tag="psS")
                        for m in range(SW // MCH):
                            nc.tensor.matmul(
                                ps[:, m * MCH : (m + 1) * MCH],
                                lhsT=kT_u[:, jt * P : (jt + 1) * P],
                                rhs=qT_u[
                                    :, sw * SW + m * MCH : sw * SW + (m + 1) * MCH
                                ],
                                start=True,
                                stop=True,
                            )
                        et = epool.tile([P, SW], bf16, tag="epool")
                        nc.scalar.activation(et, ps, EXP, scale=SCALE)
                        ets.append(et)
                    return ets

                def emit_O(u, jt, ets):
                    tiles = get_psO(u)
                    for sw in range(NSW):
                        for q2 in range(SW // QCH):
                            nc.tensor.matmul(
                                tiles[sw * (SW // QCH) + q2],
                                lhsT=v1[:, u, jt, 0:65],
                                rhs=ets[sw][:, q2 * QCH : (q2 + 1) * QCH],
                                start=(jt == 0),
                                stop=(jt == NJT - 1),
                            )

                def emit_drain(u):
                    # copies release the psO banks; normalize trails on
                    # DVE/gpsimd. For the LAST unit (critical path into proj)
                    # the copies go to ACT, which is idle after the last exp,
                    # leaving DVE free for reciprocal+multiply.
                    pb = 64 * (u % 2)
                    tiles = psO_units.pop(u)
                    o65 = obuf.tile([65, nseq], f32, tag="obuf", name=f"o_{u}")
                    rs = small.tile([1, nseq], f32, tag="rs")
                    recip = small.tile([1, nseq], f32, tag="recip")
                    bcast = small.tile([64, nseq], f32, tag="bcast")
                    cp_op = nc.vector.tensor_copy
                    for q in range(NOB):
                        # single [65,512] copy per tile releases the psO bank
                        qsl = slice(q * QCH, (q + 1) * QCH)
                        cp_op(o65[:, qsl], tiles[q])
                    for q in range(NOB):
                        qsl = slice(q * QCH, (q + 1) * QCH)
                        # rowsum row to partition 0 first: the custom-DVE
                        # reciprocal mishandles a cross-partition in/out
                        # window on hardware (fine in CoreSim)
                        cp_op(rs[:, qsl], o65[64:65, qsl])
                        nc.vector.reciprocal_approx_fast(recip[:, qsl], rs[:, qsl])
                        nc.gpsimd.partition_broadcast(bcast[:, qsl], recip[:, qsl])
                        nc.vector.tensor_mul(
                            OT[pb : pb + 64, u // 2, qsl],
                            o65[0:64, qsl],
                            bcast[:, qsl],
                        )

                prev = None
                for s in range(NU * NJT):
                    u, jt = divmod(s, NJT)
                    ets = emit_S(u, jt)
                    if prev is not None:
                        emit_O(*prev)
                        if prev[1] == NJT - 1:
                            emit_drain(prev[0])
                    prev = (u, jt, ets)
                emit_O(*prev)
                emit_drain(prev[0])

            # ======== scope C: proj partial out[i, :] ========
            with (
                tc.tile_pool(name="opool", bufs=4) as opool,
                tc.tile_pool(name="psP", bufs=4, space="PSUM") as psP,
            ):
                for it in range(NIT):
                    ps = psP.tile([P, C], f32, tag="psP")
                    # m outer: close each 256-col accumulation group before
                    # opening the next one in the same bank
                    for m in range(C // MCH):
                        for co in range(2):
                            nc.tensor.matmul(
                                ps[:, m * MCH : (m + 1) * MCH],
                                lhsT=OT[:, co, it * P : (it + 1) * P],
                                rhs=wp_sb[:, co, m * MCH : (m + 1) * MCH],
                                start=(co == 0),
                                stop=(co == 1),
                            )
                    ot = opool.tile([P, C], bf16, tag="opool")
                    cp(ot, ps)
                    # alternate issue engines: descriptor generation (~600ns)
                    # on a single engine would pace the whole proj phase
                    dma_eng = nc.sync if it % 2 == 0 else nc.scalar
                    dma_eng.dma_start(out_d[it * P : (it + 1) * P, :], ot)

    nc.compile()
    return nc


def get_nc(nseq=NSEQ):
    if nseq not in _cache:
        _cache[nseq] = _build(nseq)
    return _cache[nseq]


def make_in_maps(x, w_qkv, w_proj, nseq=NSEQ):
    import ml_dtypes

    bf = ml_dtypes.bfloat16
    x = np.ascontiguousarray(x, dtype=np.float32)
    w_qkv = np.asarray(w_qkv, dtype=np.float32)
    w_proj = np.asarray(w_proj, dtype=np.float32)
    # xT packed [p, co, n] = x[b, n, co*128+p], per batch
    xts = []
    for b in range(B):
        xt = np.ascontiguousarray(
            x[b, :nseq].T.reshape(8, P, nseq).transpose(1, 0, 2).astype(bf)
        )
        xts.append(xt)
    in_maps = []
    for core in range(8):
        b, hg = core // 4, core % 4
        hs = 4 * hg
        # wq [p, co, mt*128+f] = w_qkv[rowbase(mt)+f, co*128+p]
        wq = np.empty((P, 8, 4 * P), np.float32)
        rowbases = [hs * HD, hs * HD + P, C + hs * HD, C + hs * HD + P]
        for mt, r0 in enumerate(rowbases):
            # w_qkv[r0:r0+128, :] -> [f, c]; c = co*128+p
            blk = w_qkv[r0 : r0 + P, :].T.reshape(8, P, P).transpose(1, 0, 2)
            wq[:, :, mt * P : (mt + 1) * P] = blk
        # wv [p, co, u*64+d] = w_qkv[2C+(hs+u)*64+d, co*128+p]
        vblk = w_qkv[2 * C + hs * HD : 2 * C + (hs + 4) * HD, :]  # [256, C]
        wv = vblk.T.reshape(8, P, 4 * HD).transpose(1, 0, 2)
        # wp [p, co, e] = w_proj[e, (hs+2co)*64 + p]
        wp = np.empty((P, 2, C), np.float32)
        for co in range(2):
            c0 = (hs + 2 * co) * HD
            wp[:, co, :] = w_proj[:, c0 : c0 + P].T
        in_maps.append(
            {
                "xT": xts[b],
                "wq": np.ascontiguousarray(wq.astype(bf)),
                "wv": np.ascontiguousarray(wv.astype(bf)),
                "wp": np.ascontiguousarray(wp.astype(bf)),
            }
        )
    return in_maps


def kernel(x, w_qkv, w_proj, b_proj):
    from concourse.bass_utils import run_bass_kernel_spmd

    nc = get_nc()
    in_maps = make_in_maps(x, w_qkv, w_proj)
    res = run_bass_kernel_spmd(nc, in_maps, core_ids=list(range(8)))
    parts = [np.asarray(r["out"], dtype=np.float32) for r in res.results]
    out = np.stack(
        [
            parts[0] + parts[1] + parts[2] + parts[3],
            parts[4] + parts[5] + parts[6] + parts[7],
        ],
        axis=0,
    )
    return (out + np.asarray(b_proj, np.float32)).astype(np.float32)
